# revision 44
# baseline (speedup 1.0000x reference)
"""Trainium2 Bass kernel for nn_BaselineModelWithGNN (8-core SPMD), v2.

Self-contained: hardcodes shapes/sharding; builds, compiles and runs the Bass
program on 8 NeuronCores via the axon PJRT path.

Key observation: the reference applies each of the 3 GCN convs to the same
input x and overwrites `out`, so only conv i=2 affects the result.

v2 architecture (vs the AllGather baseline):
- Nodes are permuted on host so each graph's 2 masked nodes sit at local
  slots {0,1}: the tail's masked-node gather becomes a fixed-stride copy
  (no data-driven gather, no node-major p2 spill).
- Edges are sharded by SOURCE core: each core's y table [4096, 384] is
  private and local, so the per-edge gathers start right after the
  pre-MLPs with no cross-core exchange. This removes the 25 MB y
  AllGather (~283 us on the collective cost model) entirely.
- Each core accumulates per-dst-window partial sums over all 256 global
  windows (selector matmuls into PSUM), spilled as a [32768, 384] fp16
  partial. One 8-way ReduceScatter then delivers each core the summed
  aggregate for exactly its own 4096 nodes (output 3.1 MB -> ~94 us,
  vs 283 us for the gather-side AllGather: the collective cost model
  charges by output bytes).
- Self-loops never enter the edge list: agg += y_own after the RS.
- BatchNorm folding / u-trick for BN2 / activation-accumulated stats are
  kept from the baseline.
- The sentence branch is gated to run under the ReduceScatter, when DMA
  and PE are otherwise idle.
"""
import sys
sys.path.insert(0, "/opt/trn_rl_repo")
from contextlib import ExitStack

import numpy as np
import ml_dtypes

import bass_rust as _br
import concourse.bacc as bacc
import concourse.bass as bass
import concourse.tile as tile
from concourse import mybir
from concourse._compat import cdiv

fp32 = mybir.dt.float32
bf16 = mybir.dt.float16  # fp16 for 8x less quant noise than bf16
i16 = mybir.dt.int16
AF = mybir.ActivationFunctionType
ALU = mybir.AluOpType
AX = mybir.AxisListType

NCORES = 8
B, S, E = 64, 512, 768
D = 384
NG = 512
N = B * NG              # 32768
NEDGE = 1048576
C = 3
NPC = N // NCORES       # 4096 nodes per core
GPC = B // NCORES       # 8 graphs per core
NW = NPC // 128         # 32 windows of own nodes
NJW = 256               # global dst windows (each core covers all of them)
NCOL = 512
NCH = NPC // NCOL       # 8 column chunks
EPS = 1e-5
GCHUNK = 8              # gather chunk: tiles (of 128 edges) per dma_gather


# ---------------------------------------------------------------- BIR patch
def split_waits(nc):
    """walrus here supports ONE sync-wait per instruction; split extras onto
    NoOps inserted just before, on the same engine."""
    counter = 0
    for f in nc.m.functions:
        for bb in f.blocks:
            newlist, changed = [], False
            for inst in bb.instructions:
                si = inst.sync_info
                if si is not None and len(si.on_wait) > 1:
                    waits = list(si.on_wait)
                    for w in waits[:-1]:
                        counter += 1
                        nop = mybir.InstNoOp(name=f"I-WSPLIT-{counter}", ins=[], outs=[])
                        nop.engine = inst.engine
                        nop.sync_info = _br.SyncInfo(on_wait=[w], on_update=[])
                        newlist.append(nop)
                    inst.sync_info = _br.SyncInfo(
                        on_wait=[waits[-1]], on_update=list(si.on_update))
                    changed = True
                newlist.append(inst)
            if changed:
                bb.instructions = newlist


# ---------------------------------------------------------------- host prep
def _col3(v):
    """[384] -> [128, 3] column layout (feature f = c*128+p)."""
    return np.ascontiguousarray(np.asarray(v).reshape(3, 128).T).astype(np.float32)


def _col6(v):
    return np.ascontiguousarray(np.asarray(v).reshape(6, 128).T).astype(np.float32)


def _wchunks(w, kc, m):
    """[K, M] -> [128, kc, M] (k-chunk on partitions)."""
    K, M = w.shape
    assert K == kc * 128
    return np.ascontiguousarray(w.reshape(kc, 128, M).transpose(1, 0, 2))


def _wrap_idx(idx):
    """int16 idx array (len % 16 == 0) -> [128, len/16] dma layout."""
    blk = idx.reshape(-1, 16).T  # [16, len/16]
    return np.ascontiguousarray(np.tile(blk, (8, 1)))


def preprocess(inputs):
    mask = np.asarray(inputs["graph_masking"])
    sel = np.argsort(-mask, axis=1, kind="stable")[:, :2]
    sel = np.sort(sel, axis=1)
    ar = np.arange(NG)
    newpos = np.empty((B, NG), np.int64)
    for g in range(B):
        a, b_ = sel[g]
        rest = ar[(ar != a) & (ar != b_)]
        newpos[g, np.concatenate([[a, b_], rest])] = ar
    newid = (np.arange(B)[:, None] * NG + newpos).reshape(-1)
    orig_of_new = np.empty(N, np.int64)
    orig_of_new[newid] = np.arange(N)

    ei = np.asarray(inputs["edge_index"]).astype(np.int64)
    src = newid[ei[0]]
    dst = newid[ei[1]]

    deg = (np.bincount(dst, minlength=N) + 1.0).astype(np.float32)
    dinv = 1.0 / np.sqrt(deg)
    S1 = np.bincount(dst, weights=dinv[src], minlength=N) + dinv
    u = (dinv * S1).astype(np.float32)

    # edge -> processing core: source owner; window = global dst window
    core = src >> 12
    j = dst >> 7                                 # global window 0..255
    key = core * NJW + j
    order = np.lexsort((src, key))
    src_s, dst_s = src[order], dst[order]

    cnt = np.bincount(key, minlength=NCORES * NJW).reshape(NCORES, NJW)
    tpw = np.maximum(cdiv(cnt, 128).max(axis=0), 1).astype(np.int64)
    T_total = int(tpw.sum())
    tstart = np.zeros(NJW + 1, np.int64)
    np.cumsum(tpw, out=tstart[1:])
    kstart = np.zeros(NCORES * NJW + 1, np.int64)
    np.cumsum(cnt.reshape(-1), out=kstart[1:])

    xT = np.zeros((D, N), np.float32)
    xT[:300] = np.asarray(inputs["x_nodes"])[orig_of_new].T
    xT = xT.astype(np.float16)

    w1p = np.zeros((D, D), np.float32)
    w1p[:300] = np.asarray(inputs["w_pre1"])

    lastf = np.asarray(inputs["last_h"]).astype(np.float16)
    firstf = np.asarray(inputs["first_h"]).astype(np.float16)

    bng_g, bng_b = np.asarray(inputs["bng_g"]), np.asarray(inputs["bng_b"])
    bn_g, bn_b = np.asarray(inputs["bn_g"]), np.asarray(inputs["bn_b"])
    cols = [
        _col3(inputs["b_pre1"]), _col3(inputs["b_pre2"]),
        _col3(inputs["b_post1"]), _col3(inputs["b_post2"]),
        _col3(np.asarray(inputs["b_conv"])[2]),
        _col6(inputs["b_cat"]),
        _col3(bng_g[0]), _col3(bng_b[0]), _col3(bng_g[1]), _col3(bng_b[1]),
        _col3(bng_g[4]), _col3(bng_b[4]), _col3(bng_g[5]), _col3(bng_b[5]),
        _col3(bng_g[6]), _col3(bng_b[6]),
        _col6(bn_g[0]), _col6(bn_b[0]), _col6(bn_g[1]), _col6(bn_b[1]),
    ]
    vecs = np.concatenate(cols, axis=1)  # [128, 75]
    brow = np.zeros((1, 512), np.float32)
    brow[0, D:D + C] = np.asarray(inputs["b_out"])

    w_bf = {
        "w1": _wchunks(w1p, 3, D).astype(np.float16),
        "w2": _wchunks(np.asarray(inputs["w_pre2"]), 3, D).astype(np.float16),
        "wc": _wchunks(np.asarray(inputs["w_conv"])[2], 3, D).astype(np.float16),
        "wp1": _wchunks(np.asarray(inputs["w_post1"]), 3, D).astype(np.float16),
        "wp2": _wchunks(np.asarray(inputs["w_post2"]), 3, D).astype(np.float16),
    }
    wcat = _wchunks(np.asarray(inputs["w_cat"]), 6, E).astype(np.float16)
    wout = _wchunks(np.asarray(inputs["w_out"]), 6, C).astype(np.float32)

    in_maps = []
    for c in range(NCORES):
        n0 = c * NPC
        src_pad = np.zeros(T_total * 128, np.int64)
        dstloc = np.full(T_total * 128, -1.0, np.float32)
        for jj in range(NJW):
            k = c * NJW + jj
            a, b_ = kstart[k], kstart[k + 1]
            n = b_ - a
            pos = tstart[jj] * 128
            src_pad[pos:pos + n] = src_s[a:b_] & 4095
            dstloc[pos:pos + n] = (dst_s[a:b_] & 127).astype(np.float32)
        idx_w = _wrap_idx(src_pad.astype(np.int16))            # [128, T*8]
        dst_t = np.ascontiguousarray(dstloc.reshape(T_total, 128).T)

        deg_nm = np.ascontiguousarray(
            deg[n0:n0 + NPC].reshape(NW, 128).T)               # [128, 32]
        u_row = u[n0:n0 + NPC].reshape(1, NPC)                 # [1, 4096]

        m = {
            "lasth": np.ascontiguousarray(
                lastf[c * GPC:(c + 1) * GPC].reshape(GPC * S, E)),
            "firsth": np.ascontiguousarray(
                firstf[c * GPC:(c + 1) * GPC].reshape(GPC * S, E)),
            "xT": np.ascontiguousarray(
                xT.reshape(3, 128, N)[:, :, n0:n0 + NPC].transpose(1, 0, 2)
            ).reshape(128, 3 * NPC),
            "eidx": idx_w, "dstloc": dst_t,
            "deg": deg_nm, "urow": u_row,
            "vecs": vecs, "brow": brow,
            "w1": w_bf["w1"].reshape(128, 3 * D),
            "w2": w_bf["w2"].reshape(128, 3 * D),
            "wc": w_bf["wc"].reshape(128, 3 * D),
            "wp1": w_bf["wp1"].reshape(128, 3 * D),
            "wp2": w_bf["wp2"].reshape(128, 3 * D),
            "wcat": wcat.reshape(128, 6 * E),
            "wout": wout.reshape(128, 6 * C),
        }
        in_maps.append(m)
    meta = (tuple(int(t) for t in tpw),)
    return in_maps, meta


# ---------------------------------------------------------------- device
def build(meta, rep=1, taps=(), stage=99):
    tpw = meta[0]
    T_total = sum(tpw)
    tstart = [0]
    for t in tpw:
        tstart.append(tstart[-1] + t)

    nc = bacc.Bacc("TRN2")
    I = {}
    I["lasth"] = nc.dram_tensor("lasth", [GPC * S, E], bf16, kind="ExternalInput")
    I["firsth"] = nc.dram_tensor("firsth", [GPC * S, E], bf16, kind="ExternalInput")
    I["xT"] = nc.dram_tensor("xT", [128, 3 * NPC], bf16, kind="ExternalInput")
    I["eidx"] = nc.dram_tensor("eidx", [128, T_total * 8], i16, kind="ExternalInput")
    I["dstloc"] = nc.dram_tensor("dstloc", [128, T_total], fp32, kind="ExternalInput")
    I["deg"] = nc.dram_tensor("deg", [128, NW], fp32, kind="ExternalInput")
    I["urow"] = nc.dram_tensor("urow", [1, NPC], fp32, kind="ExternalInput")
    I["vecs"] = nc.dram_tensor("vecs", [128, 75], fp32, kind="ExternalInput")
    I["brow"] = nc.dram_tensor("brow", [1, 512], fp32, kind="ExternalInput")
    for w in ("w1", "w2", "wc", "wp1", "wp2"):
        I[w] = nc.dram_tensor(w, [128, 3 * D], bf16, kind="ExternalInput")
    I["wcat"] = nc.dram_tensor("wcat", [128, 6 * E], bf16, kind="ExternalInput")
    I["wout"] = nc.dram_tensor("wout", [128, 6 * C], fp32, kind="ExternalInput")
    outT = nc.dram_tensor("outT", [C, B], fp32, kind="ExternalOutput")
    tap_outs = {}

    grp = [list(range(NCORES))]

    with tile.TileContext(nc) as tc, ExitStack() as ctx:
        const = ctx.enter_context(tc.tile_pool(name="const", bufs=1))
        big = ctx.enter_context(tc.tile_pool(name="big", bufs=2))
        gpool = ctx.enter_context(tc.tile_pool(name="gath", bufs=3))
        spool = ctx.enter_context(tc.tile_pool(name="small", bufs=2))
        opool = ctx.enter_context(tc.tile_pool(name="once", bufs=1))
        selp = ctx.enter_context(tc.tile_pool(name="sel", bufs=4))
        hspool = ctx.enter_context(tc.tile_pool(name="hs", bufs=8))
        sqpool = ctx.enter_context(tc.tile_pool(name="sq", bufs=2))
        ppool = ctx.enter_context(tc.tile_pool(name="pt", bufs=3))
        mmps = ctx.enter_context(tc.tile_pool(name="mmps", bufs=3, space="PSUM"))
        cvps = ctx.enter_context(tc.tile_pool(name="cvps", bufs=2, space="PSUM"))
        trps = ctx.enter_context(tc.tile_pool(name="trps", bufs=2, space="PSUM"))
        typs = ctx.enter_context(tc.tile_pool(name="typs", bufs=1, space="PSUM"))
        dram = ctx.enter_context(tc.tile_pool(name="dram", bufs=1, space="DRAM"))

        # ---------------- constants
        iota = const.tile([128, 128], fp32)
        nc.gpsimd.iota(iota[:], pattern=[[1, 128]], base=0, channel_multiplier=0,
                       allow_small_or_imprecise_dtypes=True)
        pidx = const.tile([128, 1], fp32)
        nc.gpsimd.iota(pidx[:], pattern=[[0, 1]], base=0, channel_multiplier=1,
                       allow_small_or_imprecise_dtypes=True)
        ident = const.tile([128, 128], bf16)
        nc.vector.tensor_scalar(ident[:], iota[:], pidx[:], None, ALU.is_equal)

        y_own = dram.tile([NPC, D], bf16, tag="y_own")
        partial = dram.tile([N, D], bf16, tag="partial")
        agg = dram.tile([NPC, D], bf16, tag="agg")

        # head-critical loads; tail data (urow/wcat/wout/brow) deferred.
        xT_t = big.tile([128, 3, NPC], bf16, tag="big")
        nc.sync.dma_start(xT_t[:], I["xT"][:].rearrange("p (k n) -> p k n", k=3))
        deg_t = const.tile([128, NW], fp32)
        nc.sync.dma_start(deg_t[:], I["deg"][:])
        vecs = const.tile([128, 75], fp32)
        nc.sync.dma_start(vecs[:], I["vecs"][:])
        W = {}
        for w in ("w1", "w2", "wc", "wp1", "wp2"):
            W[w] = const.tile([128, 3, D], bf16, name=f"W_{w}", tag=f"W_{w}")
            nc.sync.dma_start(W[w][:], I[w][:].rearrange("p (k m) -> p k m", k=3))
        idx_t = const.tile([128, T_total * 8], i16)
        nc.sync.dma_start(idx_t[:], I["eidx"][:])
        dst_t = const.tile([128, T_total], fp32)
        nc.sync.dma_start(dst_t[:], I["dstloc"][:])
        u_row = const.tile([1, NPC], fp32)
        wcat = const.tile([128, 6, E], bf16)
        wout = const.tile([128, 6, C], fp32)
        brow = const.tile([1, 512], fp32)

        def load_deferred(gate_ap):
            """Gate tail loads behind `gate_ap` so they don't steal head DMA."""
            gate32 = const.tile([1, 1], fp32)
            nc.vector.tensor_copy(gate32[:], gate_ap)
            for ap in (u_row[0:1, 0:1], wcat[0:1, 0, 0:1], wout[0:1, 0, 0:1],
                       brow[0:1, 0:1]):
                nc.vector.tensor_copy(ap, gate32[:])
            nc.sync.dma_start(u_row[:], I["urow"][:])
            nc.sync.dma_start(wcat[:], I["wcat"][:].rearrange("p (k m) -> p k m", k=6))
            nc.sync.dma_start(wout[:], I["wout"][:].rearrange("p (k m) -> p k m", k=6))
            nc.sync.dma_start(brow[:], I["brow"][:])

        onesB = const.tile([1, B], fp32)
        nc.vector.memset(onesB[:], 1.0)
        onescol = const.tile([128, 1], bf16)
        nc.vector.memset(onescol[:], 1.0)

        VO = {}
        off = 0
        for name, w_ in [("b1", 3), ("b2", 3), ("bp1", 3), ("bp2", 3),
                         ("bcv", 3), ("bcat", 6),
                         ("g0", 3), ("be0", 3), ("g1", 3), ("be1", 3),
                         ("g4", 3), ("be4", 3), ("g5", 3), ("be5", 3),
                         ("g6", 3), ("be6", 3),
                         ("gc0", 6), ("bc0", 6), ("gc1", 6), ("bc1", 6)]:
            VO[name] = (off, w_)
            off += w_

        def vcol(name):
            o, w_ = VO[name]
            return vecs[:, o:o + w_]

        dinv = const.tile([128, NW], fp32)
        nc.scalar.sqrt(dinv[:], deg_t[:])
        nc.vector.reciprocal(dinv[:], dinv[:])

        def tap(name, ap):
            if name not in taps:
                return
            t_ = nc.dram_tensor(f"tap_{name}", list(ap.shape), ap.dtype,
                                kind="ExternalOutput")
            tap_outs[name] = t_
            nc.sync.dma_start(t_[:], ap)

        def finish_early(src_ap, width=GPC):
            fin0 = spool.tile([C, B], fp32, name="fin0", tag="fin")
            nc.vector.memset(fin0[:], 0.0)
            nc.vector.tensor_scalar(fin0[:, 0:width], src_ap, 1.0, None, ALU.mult)
            nc.sync.dma_start(outT[:], fin0[:])

        for _rep in range(rep):
            # ---------------- helpers
            def gather_stats(st, width, tag, gate_ap=None):
                """st [128, width] per-core sums -> AllGather + local sum.
                If gate_ap is given, the stats upload DMA is ordered after it
                (turns this collective into a cross-core barrier)."""
                if gate_ap is not None:
                    gz = spool.tile([1, 1], fp32, tag=tag + "_gz")
                    nc.vector.tensor_scalar(gz[:], gate_ap, 0.0, None, ALU.mult)
                    nc.vector.tensor_tensor(st[0:1, 0:1], st[0:1, 0:1], gz[:],
                                            ALU.add)
                cin = dram.tile([128, width], fp32, tag=tag + "_ci")
                cout = dram.tile([NCORES * 128, width], fp32, tag=tag + "_co")
                nc.sync.dma_start(cin[:], st)
                nc.gpsimd.collective_compute(
                    "AllGather", ALU.bypass, replica_groups=grp,
                    ins=[cin[:]], outs=[cout[:]])
                allst = opool.tile([128, NCORES, width], fp32, tag=tag + "_as")
                nc.sync.dma_start(
                    allst[:], cout[:].rearrange("(g p) c -> p g c", g=NCORES))
                av = allst[:].rearrange("p g c -> p (g c)")
                red = opool.tile([128, width], fp32, tag=tag + "_rd")
                half = opool.tile([128, 4 * width], fp32, tag=tag + "_hf")
                nc.vector.tensor_tensor(
                    half[:], av[:, 0:4 * width], av[:, 4 * width:8 * width], ALU.add)
                nc.vector.tensor_tensor(
                    half[:, 0:2 * width], half[:, 0:2 * width],
                    half[:, 2 * width:4 * width], ALU.add)
                nc.vector.tensor_tensor(
                    red[:], half[:, 0:width], half[:, width:2 * width], ALU.add)
                return red, allst

            def bn_coeffs(red, m_chunks, count, gname, bname, tag):
                gp = spool.tile([128, m_chunks], fp32, tag=tag + "_gp")
                bp = spool.tile([128, m_chunks], fp32, tag=tag + "_bp")
                mu = spool.tile([128, m_chunks], fp32, tag=tag + "_mu")
                var = spool.tile([128, m_chunks], fp32, tag=tag + "_va")
                inv_n = 1.0 / count
                sview = red.rearrange("p (m two) -> p m two", two=2)
                nc.vector.tensor_scalar(mu[:], sview[:, :, 0], inv_n, None, ALU.mult)
                nc.vector.tensor_scalar(var[:], sview[:, :, 1], inv_n, None, ALU.mult)
                musq = spool.tile([128, m_chunks], fp32, tag=tag + "_ms")
                nc.vector.tensor_mul(musq[:], mu[:], mu[:])
                nc.vector.tensor_tensor(var[:], var[:], musq[:], ALU.subtract)
                nc.vector.tensor_scalar(var[:], var[:], EPS, None, ALU.add)
                nc.scalar.sqrt(var[:], var[:])
                nc.vector.reciprocal(var[:], var[:])
                nc.vector.tensor_mul(gp[:], vcol(gname), var[:])
                nc.vector.tensor_mul(bp[:], gp[:], mu[:])
                nc.vector.tensor_tensor(bp[:], vcol(bname), bp[:], ALU.subtract)
                return gp, bp

            def fold_bn(wsrc, gp, bp, bias_name, tag):
                # bias matmuls first (read unscaled W), then scale W in place
                bpb = spool.tile([128, 3], bf16, tag=tag + "_bpb")
                nc.vector.tensor_copy(bpb[:], bp[:])
                ps_b = typs.tile([128, B], fp32, name="tinyps", tag="tiny")[:, 0:GPC]
                for m in range(3):
                    for k in range(3):
                        nc.tensor.matmul(
                            ps_b[:, m:m + 1],
                            lhsT=wsrc[:, k, m * 128:(m + 1) * 128],
                            rhs=bpb[:, k:k + 1], start=(k == 0), stop=(k == 2))
                bias = spool.tile([128, 3], fp32, tag=tag + "_bi")
                nc.vector.tensor_tensor(
                    bias[:], ps_b[:, 0:3], vcol(bias_name), ALU.add)
                for k in range(3):
                    nc.vector.tensor_scalar(
                        wsrc[:, k, :], wsrc[:, k, :], gp[:, k:k + 1], None,
                        ALU.mult)
                return wsrc, bias

            def mlp_layer(x_t, w_t, bias_ap, tag, chunk_cb=None):
                """x_t [128,3,NPC] bf16 -> relu(x@W + b) bf16 [128,3,NPC]
                + [128, 6] sum/sumsq stats via activation accum."""
                out = big.tile([128, 3, NPC], bf16, tag="big")
                sums = spool.tile([128, 3, NCH], fp32, tag=tag + "_sc")
                sqs = spool.tile([128, 3, NCH], fp32, tag=tag + "_qc")
                for jc in range(NCH):
                    if chunk_cb is not None and jc > 0:
                        chunk_cb(jc - 1, out)
                    for m in range(3):
                        bcol = bias_ap[:, m:m + 1]
                        sl = slice(jc * NCOL, (jc + 1) * NCOL)
                        ps = mmps.tile([128, NCOL], fp32, tag="mm")
                        for k in range(3):
                            nc.tensor.matmul(
                                ps[:], lhsT=w_t[:, k, m * 128:(m + 1) * 128],
                                rhs=x_t[:, k, sl], start=(k == 0), stop=(k == 2))
                        nc.scalar.activation(out[:, m, sl], ps[:], AF.Relu,
                                             bias=bcol,
                                             accum_out=sums[:, m, jc:jc + 1])
                        sq = sqpool.tile([128, NCOL], bf16, name="sqscr", tag="sqscr")
                        if m == 0:
                            nc.scalar.activation(sq[:], out[:, m, sl], AF.Square,
                                                 accum_out=sqs[:, m, jc:jc + 1])
                        else:
                            nc.vector.tensor_mul(sq[:], out[:, m, sl], out[:, m, sl])
                            nc.vector.reduce_sum(sqs[:, m, jc:jc + 1], sq[:], axis=AX.X)
                st = spool.tile([128, 6], fp32, tag=tag + "_st")
                for m in range(3):
                    nc.vector.reduce_sum(st[:, 2 * m:2 * m + 1], sums[:, m, :], axis=AX.X)
                    nc.vector.reduce_sum(st[:, 2 * m + 1:2 * m + 2], sqs[:, m, :], axis=AX.X)
                if chunk_cb is not None:
                    chunk_cb(NCH - 1, out)
                return out, st

            # ---------------- pre-MLPs
            x1, st1 = mlp_layer(xT_t, W["w1"], vcol("b1"), "l1")
            red1, _ = gather_stats(st1[:], 6, "ar1")
            gp1, bp1 = bn_coeffs(red1, 3, N, "g0", "be0", "bn1")
            w2f, bias2 = fold_bn(W["w2"], gp1, bp1, "b2", "f2")
            tap("x1", x1[:])

            # y-build interleaved into L2's chunk loop: windows 4*jc..4*jc+3
            # transpose + scale + spill as soon as chunk jc's columns land.
            ybuf = const.tile([128, NW, D], bf16)

            def ybuild_chunk(jc, x2t):
                for w in range(4 * jc, 4 * jc + 4):
                    for m in range(3):
                        trp = trps.tile([128, 128], bf16, tag="tr")
                        nc.tensor.transpose(
                            trp[:], x2t[:, m, w * 128:(w + 1) * 128], ident[:])
                        nc.vector.tensor_scalar(
                            ybuf[:, w, m * 128:(m + 1) * 128], trp[:],
                            dinv[:, w:w + 1], None, ALU.mult)
                    nc.sync.dma_start(y_own[w * 128:(w + 1) * 128, :],
                                      ybuf[:, w, :])

            x2, st2 = mlp_layer(x1, w2f, bias2, "l2", chunk_cb=ybuild_chunk)
            tap("x2", x2[:])
            if stage <= 2:
                red2d, _ = gather_stats(st2[:], 6, "ar2")
                finish_early(red2d[0:C, 0:6], width=6)
                continue

            # BN2 stats sync runs under the gather phase (u-trick: nothing
            # before the conv tail needs it)
            red2, _ = gather_stats(st2[:], 6, "ar2")
            gp2, bp2 = bn_coeffs(red2, 3, N, "g1", "be1", "bn2")
            load_deferred(ybuf[0:1, NW - 1, 0:1])

            # BN2 folds for the conv (u-trick): t2 row first, then scale wc
            # in place
            biasc = vcol("bcv")
            bpb2 = spool.tile([128, 3], bf16, tag="bpb2")
            nc.vector.tensor_copy(bpb2[:], bp2[:])
            ps_t = typs.tile([128, B], fp32, name="tinyps", tag="tiny")[:, 0:GPC]
            for m in range(3):
                for k in range(3):
                    nc.tensor.matmul(
                        ps_t[:, m:m + 1],
                        lhsT=W["wc"][:, k, m * 128:(m + 1) * 128],
                        rhs=bpb2[:, k:k + 1], start=(k == 0), stop=(k == 2))
            wcf = W["wc"]
            for k in range(3):
                nc.vector.tensor_scalar(
                    wcf[:, k, :], wcf[:, k, :], gp2[:, k:k + 1], None, ALU.mult)
            twc_col = spool.tile([128, 3], bf16, tag="twc_c")
            nc.vector.tensor_copy(twc_col[:], ps_t[:, 0:3])
            twcT = spool.tile([1, 3 * 128], bf16, tag="twcT")
            for m in range(3):
                trp_t = trps.tile([128, 128], bf16, tag="tr")
                nc.tensor.transpose(trp_t[0:1, :], twc_col[:, m:m + 1], ident[:])
                nc.vector.tensor_copy(twcT[:, m * 128:(m + 1) * 128],
                                      trp_t[0:1, :])
            u_rowh = const.tile([1, NPC], bf16)
            nc.vector.tensor_copy(u_rowh[:], u_row[:])

            # ---------------- gather phase: flat tile stream, chunks span
            # window boundaries (fewer SWDGE calls -> Pool engine relief)
            tile2win = []
            for jj in range(NJW):
                tile2win += [jj] * tpw[jj]
            pt_last = None
            ps_map = {}
            nch_f = cdiv(T_total, GCHUNK)
            base_f, extra_f = divmod(T_total, nch_f)
            t = 0
            for ci in range(nch_f):
                cn = base_f + (1 if ci < extra_f else 0)
                gt = gpool.tile([128, GCHUNK * D], bf16, tag="g")
                nc.gpsimd.dma_gather(
                    out_ap=gt[:, :cn * D].rearrange("p (t f) -> p t f", f=D),
                    in_ap=y_own[:],
                    idxs_ap=idx_t[:, t * 8:(t + cn) * 8],
                    num_idxs=cn * 128, num_idxs_reg=cn * 128, elem_size=D)
                gv = gt[:, :cn * D].rearrange("p (t f) -> p t f", f=D)
                for tl in range(cn):
                    tg = t + tl
                    jj = tile2win[tg]
                    first = (tg == tstart[jj])
                    last = (tg == tstart[jj + 1] - 1)
                    if first:
                        ps_map[jj] = cvps.tile([128, D], fp32, name="cvw", tag="cv")
                    sel = selp.tile([128, 128], bf16, tag="sel")
                    nc.vector.tensor_scalar(sel[:], iota[:], dst_t[:, tg:tg + 1],
                                            None, ALU.is_equal)
                    nc.tensor.matmul(ps_map[jj][:], lhsT=sel[:], rhs=gv[:, tl, :],
                                     start=first, stop=last)
                    if last:
                        pt = ppool.tile([128, D], bf16, tag="pt")
                        nc.scalar.activation(pt[:], ps_map.pop(jj)[:], AF.Copy)
                        nc.sync.dma_start(
                            partial[jj * 128:(jj + 1) * 128, :], pt[:])
                        pt_last = pt
                t += cn

            # ---------------- ReduceScatter of the partial aggregates
            nc.gpsimd.collective_compute(
                "ReduceScatter", ALU.add, replica_groups=grp,
                ins=[partial[:]], outs=[agg[:]])

            # ---------------- sentence branch (gated to run under the RS)
            HsT = opool.tile([128, 6, GPC], fp32, tag="HsT")
            for b in range(GPC):
                ps_ht = typs.tile([128, B], fp32, name="tinyps", tag="tiny")[:, 0:GPC]
                hts = []
                for hsrc in (I["lasth"], I["firsth"]):
                    for sc in range(4):
                        ht = hspool.tile([128, E], bf16, name="ht", tag="ht")
                        if b == 0:
                            nc.vector.tensor_copy(ht[0:1, 0:1], pt_last[0:1, 0:1])
                        nc.sync.dma_start(
                            ht[:], hsrc[b * S + sc * 128:b * S + (sc + 1) * 128, :])
                        hts.append(ht)
                for m in range(6):
                    for i, ht in enumerate(hts):
                        nc.tensor.matmul(
                            ps_ht[:, m:m + 1],
                            lhsT=ht[:, m * 128:(m + 1) * 128],
                            rhs=onescol[:],
                            start=(i == 0), stop=(i == 7))
                nc.vector.tensor_scalar(
                    HsT[:, :, b], ps_ht[:, 0:6],
                    1.0 / (2 * S), None, ALU.mult)
            tap("hsT", HsT[:])

            # ---------------- conv tail: agg + self-loop, dinv_d scale, Wc'
            convT = big.tile([128, 3, NPC], bf16, tag="big")
            csums = spool.tile([128, 3, NW], fp32, tag="cv_sc")
            csqs = spool.tile([128, 3, NW], fp32, tag="cv_qc")
            for w in range(NW):
                aw = ppool.tile([128, D], bf16, tag="aw")
                nc.sync.dma_start(aw[:], agg[w * 128:(w + 1) * 128, :])
                asum = ppool.tile([128, D], bf16, tag="asum")
                nc.vector.tensor_tensor(asum[:], aw[:], ybuf[:, w, :], ALU.add)
                aggT = spool.tile([128, 3, 128], bf16, tag="aggT")
                for m in range(3):
                    seg = asum[:, m * 128:(m + 1) * 128]
                    nc.vector.tensor_scalar(seg, seg, dinv[:, w:w + 1],
                                            None, ALU.mult)
                    trp = trps.tile([128, 128], bf16, tag="tr")
                    nc.tensor.transpose(trp[:], seg, ident[:])
                    nc.scalar.activation(aggT[:, m, :], trp[:], AF.Copy)
                for m in range(3):
                    ps2f = mmps.tile([128, NCOL], fp32, tag="mm")
                    ps2 = ps2f[:, 0:128]
                    for k in range(3):
                        nc.tensor.matmul(
                            ps2, lhsT=wcf[:, k, m * 128:(m + 1) * 128],
                            rhs=aggT[:, k, :], start=(k == 0), stop=False)
                    nc.tensor.matmul(
                        ps2, lhsT=twcT[:, m * 128:(m + 1) * 128],
                        rhs=u_rowh[:, w * 128:(w + 1) * 128],
                        start=False, stop=True)
                    nc.scalar.activation(convT[:, m, w * 128:(w + 1) * 128],
                                         ps2[:], AF.Relu,
                                         bias=biasc[:, m:m + 1],
                                         accum_out=csums[:, m, w:w + 1])
                    cseg = convT[:, m, w * 128:(w + 1) * 128]
                    if m == 0:
                        sq = sqpool.tile([128, NCOL], bf16, name="sqscr2",
                                         tag="sqscr")
                        nc.scalar.activation(sq[:, 0:128], cseg, AF.Square,
                                             accum_out=csqs[:, m, w:w + 1])
                    else:
                        sq = sqpool.tile([128, NCOL], bf16, name="sqscr2",
                                         tag="sqscr")
                        nc.vector.tensor_mul(sq[:, 0:128], cseg, cseg)
                        nc.vector.reduce_sum(csqs[:, m, w:w + 1], sq[:, 0:128],
                                             axis=AX.X)
            if stage <= 3.9:
                finish_early(convT[0:C, 0, 0:GPC])
                continue
            st4 = spool.tile([128, 6], fp32, tag="st4")
            for m in range(3):
                nc.vector.reduce_sum(st4[:, 2 * m:2 * m + 1], csums[:, m, :], axis=AX.X)
                nc.vector.reduce_sum(st4[:, 2 * m + 1:2 * m + 2], csqs[:, m, :], axis=AX.X)
            tap("convT", convT[:])
            red4, _ = gather_stats(st4[:], 6, "ar4")
            gp4, bp4 = bn_coeffs(red4, 3, N, "g4", "be4", "bn4")
            wp1f, biasp1 = fold_bn(W["wp1"], gp4, bp4, "bp1", "f4")
            if stage <= 4:
                finish_early(convT[0:C, 0, 0:GPC])
                continue

            # ---------------- post MLPs
            p1, st5 = mlp_layer(convT, wp1f, biasp1, "l5")
            red5, _ = gather_stats(st5[:], 6, "ar5")
            gp5, bp5 = bn_coeffs(red5, 3, N, "g5", "be5", "bn5")
            wp2f, biasp2 = fold_bn(W["wp2"], gp5, bp5, "bp2", "f5")

            p2, st6 = mlp_layer(p1, wp2f, biasp2, "l6")

            # combined final sync: BN6 stats + raw masked p2 + raw H_sent in
            # ONE AllGather; every core then finishes the (tiny) head for all
            # 64 graphs locally — no further collectives.
            pay = opool.tile([128, 102], fp32, tag="pay")
            nc.vector.tensor_copy(pay[:, 0:6], st6[:])
            for m in range(3):
                src_ap = p2[:, m, :].rearrange("p (g n) -> p g n", g=GPC)[:, :, 0:2]
                dst_ap = pay[:, 6 + 16 * m:6 + 16 * (m + 1)].rearrange(
                    "p (g n) -> p g n", n=2)
                nc.vector.tensor_copy(dst_ap, src_ap)
            nc.vector.tensor_copy(pay[:, 54:102],
                                  HsT[:].rearrange("p m b -> p (m b)"))
            red6, all6 = gather_stats(pay[:], 102, "ar6")
            gp6, bp6 = bn_coeffs(red6[:, 0:6], 3, N, "g6", "be6", "bn6")
            if stage <= 5:
                finish_early(red6[0:C, 0:6], width=6)
                continue

            # all-graph masked nodes [128, 3, 2B], BN6 applied; col = 2*graph+j
            flT = opool.tile([128, 3, 2 * B], bf16, tag="flT")
            for m in range(3):
                src_ap = all6[:, :, 6 + 16 * m:6 + 16 * (m + 1)]
                dst_ap = flT[:, m, :].rearrange("p (g c) -> p g c", c=16)
                nc.vector.tensor_scalar(dst_ap, src_ap,
                                        gp6[:, m:m + 1], bp6[:, m:m + 1],
                                        ALU.mult, ALU.add)
            tap("flT", flT[:])
            HsAll = opool.tile([128, 6, B], fp32, tag="HsAll")
            nc.vector.tensor_copy(
                HsAll[:].rearrange("p m (g b) -> p m g b", g=NCORES),
                all6[:, :, 54:102].rearrange("p g (m b) -> p m g b", b=GPC))

            # ---------------- local tail for all 64 graphs
            outcT = opool.tile([128, 6, B], fp32, tag="outcT")
            for m in range(6):
                ps_o = typs.tile([128, B], fp32, tag="tiny")
                for k in range(6):
                    kc, kj = k % 3, k // 3
                    nc.tensor.matmul(
                        ps_o[:], lhsT=wcat[:, k, m * 128:(m + 1) * 128],
                        rhs=flT[:, kc, kj::2], start=(k == 0), stop=(k == 5))
                nc.scalar.activation(outcT[:, m, :], ps_o[:], AF.Relu,
                                     bias=vcol("bcat")[:, m:m + 1])
            stt = spool.tile([128, 24], fp32, tag="stt")
            for m in range(6):
                nc.vector.reduce_sum(stt[:, 2 * m:2 * m + 1], outcT[:, m, :], axis=AX.X)
                sq = spool.tile([128, B], fp32, tag="ttsq")
                nc.scalar.square(sq[:], outcT[:, m, :])
                nc.vector.reduce_sum(stt[:, 2 * m + 1:2 * m + 2], sq[:], axis=AX.X)
                nc.vector.reduce_sum(stt[:, 12 + 2 * m:13 + 2 * m], HsAll[:, m, :], axis=AX.X)
                nc.scalar.square(sq[:], HsAll[:, m, :])
                nc.vector.reduce_sum(stt[:, 13 + 2 * m:14 + 2 * m], sq[:], axis=AX.X)
            gpc_, bpc_ = bn_coeffs(stt[:, 0:12], 6, B, "gc0", "bc0", "bnc")
            gph, bph = bn_coeffs(stt[:, 12:24], 6, B, "gc1", "bc1", "bnh")
            attT = opool.tile([128, 6, B], fp32, tag="attT")
            for m in range(6):
                nc.vector.tensor_scalar(attT[:, m, :], HsAll[:, m, :],
                                        gph[:, m:m + 1], bph[:, m:m + 1],
                                        ALU.mult, ALU.add)
                nc.vector.tensor_scalar(outcT[:, m, :], outcT[:, m, :],
                                        gpc_[:, m:m + 1], bpc_[:, m:m + 1],
                                        ALU.mult, ALU.add)
                nc.vector.tensor_add(attT[:, m, :], attT[:, m, :], outcT[:, m, :])
            ps_ft = typs.tile([128, B], fp32, tag="tiny")
            ps_f = ps_ft[0:C, :]
            for k in range(6):
                nc.tensor.matmul(ps_f, lhsT=wout[:, k, :], rhs=attT[:, k, :],
                                 start=(k == 0), stop=False)
            nc.tensor.matmul(ps_f, lhsT=brow[0:1, D:D + C], rhs=onesB[:],
                             start=False, stop=True)
            fin = spool.tile([C, B], fp32, tag="fin")
            nc.vector.tensor_copy(fin[:], ps_f)
            nc.sync.dma_start(outT[:], fin[:])

    nc.compile()
    return nc, tap_outs


# ---------------------------------------------------------------- entry
_CACHE = {}


def _get_compiled(meta):
    key = meta
    if key not in _CACHE:
        nc, _ = build(meta)
        split_waits(nc)
        _CACHE[key] = nc
    return _CACHE[key]


def kernel(**inputs):
    in_maps, meta = preprocess(inputs)
    nc = _get_compiled(meta)
    from concourse import bass2jax
    results = bass2jax.run_bass_via_pjrt(nc, in_maps, n_cores=NCORES)
    out = np.concatenate(
        [results[c]["outT"].T[c * GPC:(c + 1) * GPC] for c in range(NCORES)],
        axis=0)
    return out.astype(np.float32)


# revision 53
# speedup vs baseline: 1.1702x; 1.1702x over previous
"""Trainium2 Bass kernel for nn_BaselineModelWithGNN (8-core SPMD), v2.

Self-contained: hardcodes shapes/sharding; builds, compiles and runs the Bass
program on 8 NeuronCores via the axon PJRT path.

Key observation: the reference applies each of the 3 GCN convs to the same
input x and overwrites `out`, so only conv i=2 affects the result.

v2 architecture (vs the AllGather baseline):
- Nodes are permuted on host so each graph's 2 masked nodes sit at local
  slots {0,1}: the tail's masked-node gather becomes a fixed-stride copy
  (no data-driven gather, no node-major p2 spill).
- Edges are sharded by SOURCE core: each core's y table [4096, 384] is
  private and local, so the per-edge gathers start right after the
  pre-MLPs with no cross-core exchange. This removes the 25 MB y
  AllGather (~283 us on the collective cost model) entirely.
- Each core accumulates per-dst-window partial sums over all 256 global
  windows (selector matmuls into PSUM), spilled as a [32768, 384] fp16
  partial. One 8-way ReduceScatter then delivers each core the summed
  aggregate for exactly its own 4096 nodes (output 3.1 MB -> ~94 us,
  vs 283 us for the gather-side AllGather: the collective cost model
  charges by output bytes).
- Self-loops never enter the edge list: agg += y_own after the RS.
- BatchNorm folding / u-trick for BN2 / activation-accumulated stats are
  kept from the baseline.
- The sentence branch is gated to run under the ReduceScatter, when DMA
  and PE are otherwise idle.
"""
import sys
sys.path.insert(0, "/opt/trn_rl_repo")
from contextlib import ExitStack

import numpy as np
import ml_dtypes

import bass_rust as _br
import concourse.bacc as bacc
import concourse.bass as bass
import concourse.tile as tile
from concourse import mybir
from concourse._compat import cdiv

fp32 = mybir.dt.float32
bf16 = mybir.dt.float16  # fp16 for 8x less quant noise than bf16
i16 = mybir.dt.int16
AF = mybir.ActivationFunctionType
ALU = mybir.AluOpType
AX = mybir.AxisListType

NCORES = 8
B, S, E = 64, 512, 768
D = 384
NG = 512
N = B * NG              # 32768
NEDGE = 1048576
C = 3
NPC = N // NCORES       # 4096 nodes per core
GPC = B // NCORES       # 8 graphs per core
NW = NPC // 128         # 32 windows of own nodes
NJW = 256               # global dst windows (each core covers all of them)
NCOL = 512
NCH = NPC // NCOL       # 8 column chunks
EPS = 1e-5
GCHUNK = 8              # gather chunk: tiles (of 128 edges) per dma_gather


# ---------------------------------------------------------------- BIR patch
def split_waits(nc):
    """walrus here supports ONE sync-wait per instruction; split extras onto
    NoOps inserted just before, on the same engine."""
    counter = 0
    for f in nc.m.functions:
        for bb in f.blocks:
            newlist, changed = [], False
            for inst in bb.instructions:
                si = inst.sync_info
                if si is not None and len(si.on_wait) > 1:
                    waits = list(si.on_wait)
                    for w in waits[:-1]:
                        counter += 1
                        nop = mybir.InstNoOp(name=f"I-WSPLIT-{counter}", ins=[], outs=[])
                        nop.engine = inst.engine
                        nop.sync_info = _br.SyncInfo(on_wait=[w], on_update=[])
                        newlist.append(nop)
                    inst.sync_info = _br.SyncInfo(
                        on_wait=[waits[-1]], on_update=list(si.on_update))
                    changed = True
                newlist.append(inst)
            if changed:
                bb.instructions = newlist


# ---------------------------------------------------------------- host prep
def _col3(v):
    """[384] -> [128, 3] column layout (feature f = c*128+p)."""
    return np.ascontiguousarray(np.asarray(v).reshape(3, 128).T).astype(np.float32)


def _col6(v):
    return np.ascontiguousarray(np.asarray(v).reshape(6, 128).T).astype(np.float32)


def _wchunks(w, kc, m):
    """[K, M] -> [128, kc, M] (k-chunk on partitions)."""
    K, M = w.shape
    assert K == kc * 128
    return np.ascontiguousarray(w.reshape(kc, 128, M).transpose(1, 0, 2))


def _wrap_idx(idx):
    """int16 idx array (len % 16 == 0) -> [128, len/16] dma layout."""
    blk = idx.reshape(-1, 16).T  # [16, len/16]
    return np.ascontiguousarray(np.tile(blk, (8, 1)))


def preprocess(inputs):
    mask = np.asarray(inputs["graph_masking"])
    sel = np.argsort(-mask, axis=1, kind="stable")[:, :2]
    sel = np.sort(sel, axis=1)
    ar = np.arange(NG)
    newpos = np.empty((B, NG), np.int64)
    for g in range(B):
        a, b_ = sel[g]
        rest = ar[(ar != a) & (ar != b_)]
        newpos[g, np.concatenate([[a, b_], rest])] = ar
    newid = (np.arange(B)[:, None] * NG + newpos).reshape(-1)
    orig_of_new = np.empty(N, np.int64)
    orig_of_new[newid] = np.arange(N)

    ei = np.asarray(inputs["edge_index"]).astype(np.int64)
    src = newid[ei[0]]
    dst = newid[ei[1]]

    deg = (np.bincount(dst, minlength=N) + 1.0).astype(np.float32)
    dinv = 1.0 / np.sqrt(deg)
    S1 = np.bincount(dst, weights=dinv[src], minlength=N) + dinv
    u = (dinv * S1).astype(np.float32)

    # edge -> processing core: source owner; window = global dst window
    core = src >> 12
    j = dst >> 7                                 # global window 0..255
    key = core * NJW + j
    order = np.lexsort((src, key))
    src_s, dst_s = src[order], dst[order]

    cnt = np.bincount(key, minlength=NCORES * NJW).reshape(NCORES, NJW)
    kstart = np.zeros(NCORES * NJW + 1, np.int64)
    np.cumsum(cnt.reshape(-1), out=kstart[1:])

    # shared Lindley slot schedule: window jj's tail may spill into the
    # FIRST tile of slot jj+1 (that boundary tile runs a second, spill
    # selector matmul closing window jj). Removes most ceil padding.
    Lrun = np.zeros(NCORES, np.int64)
    tiles = np.zeros(NJW, np.int64)
    start_in = np.zeros((NCORES, NJW), np.int64)
    for jj in range(NJW):
        need = Lrun + cnt[:, jj]
        if jj == NJW - 1:
            t_ = max(1, int(-(-int(need.max()) // 128)))
        else:
            t_ = max(1, int(need.max()) // 128)
        tiles[jj] = t_
        start_in[:, jj] = Lrun
        Lrun = np.maximum(0, need - t_ * 128)
    assert int(Lrun.max()) == 0
    tpw = tiles
    T_total = int(tiles.sum())
    tstart = np.zeros(NJW + 1, np.int64)
    np.cumsum(tiles, out=tstart[1:])

    xT = np.zeros((D, N), np.float32)
    xT[:300] = np.asarray(inputs["x_nodes"])[orig_of_new].T
    xT = xT.astype(np.float16)

    w1p = np.zeros((D, D), np.float32)
    w1p[:300] = np.asarray(inputs["w_pre1"])

    lastf = np.asarray(inputs["last_h"]).astype(np.float16)
    firstf = np.asarray(inputs["first_h"]).astype(np.float16)

    bng_g, bng_b = np.asarray(inputs["bng_g"]), np.asarray(inputs["bng_b"])
    bn_g, bn_b = np.asarray(inputs["bn_g"]), np.asarray(inputs["bn_b"])
    cols = [
        _col3(inputs["b_pre1"]), _col3(inputs["b_pre2"]),
        _col3(inputs["b_post1"]), _col3(inputs["b_post2"]),
        _col3(np.asarray(inputs["b_conv"])[2]),
        _col6(inputs["b_cat"]),
        _col3(bng_g[0]), _col3(bng_b[0]), _col3(bng_g[1]), _col3(bng_b[1]),
        _col3(bng_g[4]), _col3(bng_b[4]), _col3(bng_g[5]), _col3(bng_b[5]),
        _col3(bng_g[6]), _col3(bng_b[6]),
        _col6(bn_g[0]), _col6(bn_b[0]), _col6(bn_g[1]), _col6(bn_b[1]),
    ]
    vecs = np.concatenate(cols, axis=1)  # [128, 75]
    brow = np.zeros((1, 512), np.float32)
    brow[0, D:D + C] = np.asarray(inputs["b_out"])

    w_bf = {
        "w1": _wchunks(w1p, 3, D).astype(np.float16),
        "w2": _wchunks(np.asarray(inputs["w_pre2"]), 3, D).astype(np.float16),
        "wc": _wchunks(np.asarray(inputs["w_conv"])[2], 3, D).astype(np.float16),
        "wp1": _wchunks(np.asarray(inputs["w_post1"]), 3, D).astype(np.float16),
        "wp2": _wchunks(np.asarray(inputs["w_post2"]), 3, D).astype(np.float16),
    }
    wcat = _wchunks(np.asarray(inputs["w_cat"]), 6, E).astype(np.float16)
    wout = _wchunks(np.asarray(inputs["w_out"]), 6, C).astype(np.float32)

    in_maps = []
    for c in range(NCORES):
        n0 = c * NPC
        src_pad = np.zeros(T_total * 128, np.int64)
        dstM = np.full(T_total * 128, -1.0, np.float32)
        dstS = np.full(NJW * 128, -1.0, np.float32)
        for jj in range(NJW):
            k = c * NJW + jj
            a, b_ = kstart[k], kstart[k + 1]
            eN = b_ - a
            pos0 = tstart[jj] * 128 + start_in[c, jj]
            slot_end = tstart[jj + 1] * 128
            n_main = min(eN, slot_end - pos0)
            src_pad[pos0:pos0 + n_main] = src_s[a:a + n_main] & 4095
            dstM[pos0:pos0 + n_main] = (dst_s[a:a + n_main] & 127)
            sp = eN - n_main
            if sp > 0:
                src_pad[slot_end:slot_end + sp] = src_s[a + n_main:b_] & 4095
                dstS[jj * 128:jj * 128 + sp] = (dst_s[a + n_main:b_] & 127)
        idx_w = _wrap_idx(src_pad.astype(np.int16))            # [128, T*8]
        dst_t = np.ascontiguousarray(dstM.reshape(T_total, 128).T)
        dsp_t = np.ascontiguousarray(dstS.reshape(NJW, 128).T)

        deg_nm = np.ascontiguousarray(
            deg[n0:n0 + NPC].reshape(NW, 128).T)               # [128, 32]
        u_row = u[n0:n0 + NPC].reshape(1, NPC)                 # [1, 4096]

        m = {
            "lasth": np.ascontiguousarray(
                lastf[c * GPC:(c + 1) * GPC].reshape(GPC * S, E)),
            "firsth": np.ascontiguousarray(
                firstf[c * GPC:(c + 1) * GPC].reshape(GPC * S, E)),
            "xT": np.ascontiguousarray(
                xT.reshape(3, 128, N)[:, :, n0:n0 + NPC].transpose(1, 0, 2)
            ).reshape(128, 3 * NPC),
            "eidx": idx_w, "dstloc": dst_t, "dsps": dsp_t,
            "deg": deg_nm, "urow": u_row,
            "vecs": vecs, "brow": brow,
            "w1": w_bf["w1"].reshape(128, 3 * D),
            "w2": w_bf["w2"].reshape(128, 3 * D),
            "wc": w_bf["wc"].reshape(128, 3 * D),
            "wp1": w_bf["wp1"].reshape(128, 3 * D),
            "wp2": w_bf["wp2"].reshape(128, 3 * D),
            "wcat": wcat.reshape(128, 6 * E),
            "wout": wout.reshape(128, 6 * C),
        }
        in_maps.append(m)
    meta = (tuple(int(t) for t in tpw),)
    return in_maps, meta


# ---------------------------------------------------------------- device
def build(meta, rep=1, taps=(), stage=99):
    tpw = meta[0]
    T_total = sum(tpw)
    tstart = [0]
    for t in tpw:
        tstart.append(tstart[-1] + t)

    nc = bacc.Bacc("TRN2")
    I = {}
    I["lasth"] = nc.dram_tensor("lasth", [GPC * S, E], bf16, kind="ExternalInput")
    I["firsth"] = nc.dram_tensor("firsth", [GPC * S, E], bf16, kind="ExternalInput")
    I["xT"] = nc.dram_tensor("xT", [128, 3 * NPC], bf16, kind="ExternalInput")
    I["eidx"] = nc.dram_tensor("eidx", [128, T_total * 8], i16, kind="ExternalInput")
    I["dstloc"] = nc.dram_tensor("dstloc", [128, T_total], fp32, kind="ExternalInput")
    I["dsps"] = nc.dram_tensor("dsps", [128, NJW], fp32, kind="ExternalInput")
    I["deg"] = nc.dram_tensor("deg", [128, NW], fp32, kind="ExternalInput")
    I["urow"] = nc.dram_tensor("urow", [1, NPC], fp32, kind="ExternalInput")
    I["vecs"] = nc.dram_tensor("vecs", [128, 75], fp32, kind="ExternalInput")
    I["brow"] = nc.dram_tensor("brow", [1, 512], fp32, kind="ExternalInput")
    for w in ("w1", "w2", "wc", "wp1", "wp2"):
        I[w] = nc.dram_tensor(w, [128, 3 * D], bf16, kind="ExternalInput")
    I["wcat"] = nc.dram_tensor("wcat", [128, 6 * E], bf16, kind="ExternalInput")
    I["wout"] = nc.dram_tensor("wout", [128, 6 * C], fp32, kind="ExternalInput")
    outT = nc.dram_tensor("outT", [C, B], fp32, kind="ExternalOutput")
    tap_outs = {}

    grp = [list(range(NCORES))]

    with tile.TileContext(nc) as tc, ExitStack() as ctx:
        const = ctx.enter_context(tc.tile_pool(name="const", bufs=1))
        big = ctx.enter_context(tc.tile_pool(name="big", bufs=2))
        gpool = ctx.enter_context(tc.tile_pool(name="gath", bufs=3))
        spool = ctx.enter_context(tc.tile_pool(name="small", bufs=2))
        opool = ctx.enter_context(tc.tile_pool(name="once", bufs=1))
        selp = ctx.enter_context(tc.tile_pool(name="sel", bufs=4))
        hspool = ctx.enter_context(tc.tile_pool(name="hs", bufs=8))
        sqpool = ctx.enter_context(tc.tile_pool(name="sq", bufs=2))
        ppool = ctx.enter_context(tc.tile_pool(name="pt", bufs=3))
        mmps = ctx.enter_context(tc.tile_pool(name="mmps", bufs=3, space="PSUM"))
        cvps = ctx.enter_context(tc.tile_pool(name="cvps", bufs=2, space="PSUM"))
        trps = ctx.enter_context(tc.tile_pool(name="trps", bufs=2, space="PSUM"))
        typs = ctx.enter_context(tc.tile_pool(name="typs", bufs=1, space="PSUM"))
        dram = ctx.enter_context(tc.tile_pool(name="dram", bufs=1, space="DRAM"))

        # ---------------- constants
        iota = const.tile([128, 128], fp32)
        nc.gpsimd.iota(iota[:], pattern=[[1, 128]], base=0, channel_multiplier=0,
                       allow_small_or_imprecise_dtypes=True)
        pidx = const.tile([128, 1], fp32)
        nc.gpsimd.iota(pidx[:], pattern=[[0, 1]], base=0, channel_multiplier=1,
                       allow_small_or_imprecise_dtypes=True)
        ident = const.tile([128, 128], bf16)
        nc.vector.tensor_scalar(ident[:], iota[:], pidx[:], None, ALU.is_equal)

        y_own = dram.tile([NPC, D], bf16, tag="y_own")
        partial = dram.tile([N, D], bf16, tag="partial")
        agg = dram.tile([NPC, D], bf16, tag="agg")

        # head-critical loads; tail data (urow/wcat/wout/brow) deferred.
        xT_t = big.tile([128, 3, NPC], bf16, tag="big")
        nc.sync.dma_start(xT_t[:], I["xT"][:].rearrange("p (k n) -> p k n", k=3))
        deg_t = const.tile([128, NW], fp32)
        nc.sync.dma_start(deg_t[:], I["deg"][:])
        vecs = const.tile([128, 75], fp32)
        nc.sync.dma_start(vecs[:], I["vecs"][:])
        W = {}
        for w in ("w1", "w2", "wc", "wp1", "wp2"):
            W[w] = const.tile([128, 3, D], bf16, name=f"W_{w}", tag=f"W_{w}")
            nc.sync.dma_start(W[w][:], I[w][:].rearrange("p (k m) -> p k m", k=3))
        idx_t = const.tile([128, T_total * 8], i16)
        nc.sync.dma_start(idx_t[:], I["eidx"][:])
        dst_t = const.tile([128, T_total], fp32)
        nc.sync.dma_start(dst_t[:], I["dstloc"][:])
        dsp_t = const.tile([128, NJW], fp32)
        nc.sync.dma_start(dsp_t[:], I["dsps"][:])
        u_row = const.tile([1, NPC], fp32)
        wcat = const.tile([128, 6, E], bf16)
        wout = const.tile([128, 6, C], fp32)
        brow = const.tile([1, 512], fp32)

        def load_deferred(gate_ap):
            """Gate tail loads behind `gate_ap` so they don't steal head DMA."""
            gate32 = const.tile([1, 1], fp32)
            nc.vector.tensor_copy(gate32[:], gate_ap)
            for ap in (u_row[0:1, 0:1], wcat[0:1, 0, 0:1], wout[0:1, 0, 0:1],
                       brow[0:1, 0:1]):
                nc.vector.tensor_copy(ap, gate32[:])
            nc.sync.dma_start(u_row[:], I["urow"][:])
            nc.sync.dma_start(wcat[:], I["wcat"][:].rearrange("p (k m) -> p k m", k=6))
            nc.sync.dma_start(wout[:], I["wout"][:].rearrange("p (k m) -> p k m", k=6))
            nc.sync.dma_start(brow[:], I["brow"][:])

        onesB = const.tile([1, B], fp32)
        nc.vector.memset(onesB[:], 1.0)
        onescol = const.tile([128, 1], bf16)
        nc.vector.memset(onescol[:], 1.0)

        VO = {}
        off = 0
        for name, w_ in [("b1", 3), ("b2", 3), ("bp1", 3), ("bp2", 3),
                         ("bcv", 3), ("bcat", 6),
                         ("g0", 3), ("be0", 3), ("g1", 3), ("be1", 3),
                         ("g4", 3), ("be4", 3), ("g5", 3), ("be5", 3),
                         ("g6", 3), ("be6", 3),
                         ("gc0", 6), ("bc0", 6), ("gc1", 6), ("bc1", 6)]:
            VO[name] = (off, w_)
            off += w_

        def vcol(name):
            o, w_ = VO[name]
            return vecs[:, o:o + w_]

        dinv = const.tile([128, NW], fp32)
        nc.scalar.sqrt(dinv[:], deg_t[:])
        nc.vector.reciprocal(dinv[:], dinv[:])

        def tap(name, ap):
            if name not in taps:
                return
            t_ = nc.dram_tensor(f"tap_{name}", list(ap.shape), ap.dtype,
                                kind="ExternalOutput")
            tap_outs[name] = t_
            nc.sync.dma_start(t_[:], ap)

        def finish_early(src_ap, width=GPC):
            fin0 = spool.tile([C, B], fp32, name="fin0", tag="fin")
            nc.vector.memset(fin0[:], 0.0)
            nc.vector.tensor_scalar(fin0[:, 0:width], src_ap, 1.0, None, ALU.mult)
            nc.sync.dma_start(outT[:], fin0[:])

        for _rep in range(rep):
            # ---------------- helpers
            def gather_stats(st, width, tag, gate_ap=None):
                """st [128, width] per-core sums -> AllGather + local sum.
                If gate_ap is given, the stats upload DMA is ordered after it
                (turns this collective into a cross-core barrier)."""
                if gate_ap is not None:
                    gz = spool.tile([1, 1], fp32, tag=tag + "_gz")
                    nc.vector.tensor_scalar(gz[:], gate_ap, 0.0, None, ALU.mult)
                    nc.vector.tensor_tensor(st[0:1, 0:1], st[0:1, 0:1], gz[:],
                                            ALU.add)
                cin = dram.tile([128, width], fp32, tag=tag + "_ci")
                cout = dram.tile([NCORES * 128, width], fp32, tag=tag + "_co")
                nc.sync.dma_start(cin[:], st)
                nc.gpsimd.collective_compute(
                    "AllGather", ALU.bypass, replica_groups=grp,
                    ins=[cin[:]], outs=[cout[:]])
                allst = opool.tile([128, NCORES, width], fp32, tag=tag + "_as")
                nc.sync.dma_start(
                    allst[:], cout[:].rearrange("(g p) c -> p g c", g=NCORES))
                av = allst[:].rearrange("p g c -> p (g c)")
                red = opool.tile([128, width], fp32, tag=tag + "_rd")
                half = opool.tile([128, 4 * width], fp32, tag=tag + "_hf")
                nc.vector.tensor_tensor(
                    half[:], av[:, 0:4 * width], av[:, 4 * width:8 * width], ALU.add)
                nc.vector.tensor_tensor(
                    half[:, 0:2 * width], half[:, 0:2 * width],
                    half[:, 2 * width:4 * width], ALU.add)
                nc.vector.tensor_tensor(
                    red[:], half[:, 0:width], half[:, width:2 * width], ALU.add)
                return red, allst

            def bn_coeffs(red, m_chunks, count, gname, bname, tag):
                gp = spool.tile([128, m_chunks], fp32, tag=tag + "_gp")
                bp = spool.tile([128, m_chunks], fp32, tag=tag + "_bp")
                mu = spool.tile([128, m_chunks], fp32, tag=tag + "_mu")
                var = spool.tile([128, m_chunks], fp32, tag=tag + "_va")
                inv_n = 1.0 / count
                sview = red.rearrange("p (m two) -> p m two", two=2)
                nc.vector.tensor_scalar(mu[:], sview[:, :, 0], inv_n, None, ALU.mult)
                nc.vector.tensor_scalar(var[:], sview[:, :, 1], inv_n, None, ALU.mult)
                musq = spool.tile([128, m_chunks], fp32, tag=tag + "_ms")
                nc.vector.tensor_mul(musq[:], mu[:], mu[:])
                nc.vector.tensor_tensor(var[:], var[:], musq[:], ALU.subtract)
                nc.vector.tensor_scalar(var[:], var[:], EPS, None, ALU.add)
                nc.scalar.sqrt(var[:], var[:])
                nc.vector.reciprocal(var[:], var[:])
                nc.vector.tensor_mul(gp[:], vcol(gname), var[:])
                nc.vector.tensor_mul(bp[:], gp[:], mu[:])
                nc.vector.tensor_tensor(bp[:], vcol(bname), bp[:], ALU.subtract)
                return gp, bp

            def fold_bn(wsrc, gp, bp, bias_name, tag):
                # bias matmuls first (read unscaled W), then scale W in place
                bpb = spool.tile([128, 3], bf16, tag=tag + "_bpb")
                nc.vector.tensor_copy(bpb[:], bp[:])
                ps_b = typs.tile([128, B], fp32, name="tinyps", tag="tiny")[:, 0:GPC]
                for m in range(3):
                    for k in range(3):
                        nc.tensor.matmul(
                            ps_b[:, m:m + 1],
                            lhsT=wsrc[:, k, m * 128:(m + 1) * 128],
                            rhs=bpb[:, k:k + 1], start=(k == 0), stop=(k == 2))
                bias = spool.tile([128, 3], fp32, tag=tag + "_bi")
                nc.vector.tensor_tensor(
                    bias[:], ps_b[:, 0:3], vcol(bias_name), ALU.add)
                for k in range(3):
                    nc.vector.tensor_scalar(
                        wsrc[:, k, :], wsrc[:, k, :], gp[:, k:k + 1], None,
                        ALU.mult)
                return wsrc, bias

            def mlp_layer(x_t, w_t, bias_ap, tag, chunk_cb=None):
                """x_t [128,3,NPC] bf16 -> relu(x@W + b) bf16 [128,3,NPC]
                + [128, 6] sum/sumsq stats via activation accum."""
                out = big.tile([128, 3, NPC], bf16, tag="big")
                sums = spool.tile([128, 3, NCH], fp32, tag=tag + "_sc")
                sqs = spool.tile([128, 3, NCH], fp32, tag=tag + "_qc")
                for jc in range(NCH):
                    if chunk_cb is not None and jc > 0:
                        chunk_cb(jc - 1, out)
                    for m in range(3):
                        bcol = bias_ap[:, m:m + 1]
                        sl = slice(jc * NCOL, (jc + 1) * NCOL)
                        ps = mmps.tile([128, NCOL], fp32, tag="mm")
                        for k in range(3):
                            nc.tensor.matmul(
                                ps[:], lhsT=w_t[:, k, m * 128:(m + 1) * 128],
                                rhs=x_t[:, k, sl], start=(k == 0), stop=(k == 2))
                        nc.scalar.activation(out[:, m, sl], ps[:], AF.Relu,
                                             bias=bcol,
                                             accum_out=sums[:, m, jc:jc + 1])
                        sq = sqpool.tile([128, NCOL], bf16, name="sqscr", tag="sqscr")
                        if m == 0:
                            nc.scalar.activation(sq[:], out[:, m, sl], AF.Square,
                                                 accum_out=sqs[:, m, jc:jc + 1])
                        else:
                            nc.vector.tensor_mul(sq[:], out[:, m, sl], out[:, m, sl])
                            nc.vector.reduce_sum(sqs[:, m, jc:jc + 1], sq[:], axis=AX.X)
                st = spool.tile([128, 6], fp32, tag=tag + "_st")
                for m in range(3):
                    nc.vector.reduce_sum(st[:, 2 * m:2 * m + 1], sums[:, m, :], axis=AX.X)
                    nc.vector.reduce_sum(st[:, 2 * m + 1:2 * m + 2], sqs[:, m, :], axis=AX.X)
                if chunk_cb is not None:
                    chunk_cb(NCH - 1, out)
                return out, st

            # ---------------- pre-MLPs
            x1, st1 = mlp_layer(xT_t, W["w1"], vcol("b1"), "l1")
            red1, _ = gather_stats(st1[:], 6, "ar1")
            gp1, bp1 = bn_coeffs(red1, 3, N, "g0", "be0", "bn1")
            w2f, bias2 = fold_bn(W["w2"], gp1, bp1, "b2", "f2")
            tap("x1", x1[:])

            # y-build interleaved into L2's chunk loop: windows 4*jc..4*jc+3
            # transpose + scale + spill as soon as chunk jc's columns land.
            ybuf = const.tile([128, NW, D], bf16)

            def ybuild_chunk(jc, x2t):
                for w in range(4 * jc, 4 * jc + 4):
                    for m in range(3):
                        trp = trps.tile([128, 128], bf16, tag="tr")
                        nc.tensor.transpose(
                            trp[:], x2t[:, m, w * 128:(w + 1) * 128], ident[:])
                        nc.vector.tensor_scalar(
                            ybuf[:, w, m * 128:(m + 1) * 128], trp[:],
                            dinv[:, w:w + 1], None, ALU.mult)
                    nc.sync.dma_start(y_own[w * 128:(w + 1) * 128, :],
                                      ybuf[:, w, :])

            x2, st2 = mlp_layer(x1, w2f, bias2, "l2", chunk_cb=ybuild_chunk)
            tap("x2", x2[:])
            if stage <= 2:
                red2d, _ = gather_stats(st2[:], 6, "ar2")
                finish_early(red2d[0:C, 0:6], width=6)
                continue

            # BN2 stats sync runs under the gather phase (u-trick: nothing
            # before the conv tail needs it)
            red2, _ = gather_stats(st2[:], 6, "ar2")
            gp2, bp2 = bn_coeffs(red2, 3, N, "g1", "be1", "bn2")
            load_deferred(ybuf[0:1, NW - 1, 0:1])

            # BN2 folds for the conv (u-trick): t2 row first, then scale wc
            # in place
            biasc = vcol("bcv")
            bpb2 = spool.tile([128, 3], bf16, tag="bpb2")
            nc.vector.tensor_copy(bpb2[:], bp2[:])
            ps_t = typs.tile([128, B], fp32, name="tinyps", tag="tiny")[:, 0:GPC]
            for m in range(3):
                for k in range(3):
                    nc.tensor.matmul(
                        ps_t[:, m:m + 1],
                        lhsT=W["wc"][:, k, m * 128:(m + 1) * 128],
                        rhs=bpb2[:, k:k + 1], start=(k == 0), stop=(k == 2))
            wcf = W["wc"]
            for k in range(3):
                nc.vector.tensor_scalar(
                    wcf[:, k, :], wcf[:, k, :], gp2[:, k:k + 1], None, ALU.mult)
            twc_col = spool.tile([128, 3], bf16, tag="twc_c")
            nc.vector.tensor_copy(twc_col[:], ps_t[:, 0:3])
            twcT = spool.tile([1, 3 * 128], bf16, tag="twcT")
            for m in range(3):
                trp_t = trps.tile([128, 128], bf16, tag="tr")
                nc.tensor.transpose(trp_t[0:1, :], twc_col[:, m:m + 1], ident[:])
                nc.vector.tensor_copy(twcT[:, m * 128:(m + 1) * 128],
                                      trp_t[0:1, :])
            u_rowh = const.tile([1, NPC], bf16)
            nc.vector.tensor_copy(u_rowh[:], u_row[:])

            # ---------------- gather phase: flat Lindley-packed tile stream.
            # Window jj's tail may spill into the first tile of slot jj+1;
            # that boundary tile runs a second (spill-selector) matmul that
            # closes window jj.
            tile2slot = []
            for jj in range(NJW):
                tile2slot += [jj] * tpw[jj]

            def evict_win(jj, ps):
                pt = ppool.tile([128, D], bf16, tag="pt")
                nc.scalar.activation(pt[:], ps[:], AF.Copy)
                nc.sync.dma_start(partial[jj * 128:(jj + 1) * 128, :], pt[:])
                return pt

            pt_last = None
            ps_map = {}
            nch_f = cdiv(T_total, GCHUNK)
            base_f, extra_f = divmod(T_total, nch_f)
            t = 0
            for ci in range(nch_f):
                cn = base_f + (1 if ci < extra_f else 0)
                gt = gpool.tile([128, GCHUNK * D], bf16, tag="g")
                nc.gpsimd.dma_gather(
                    out_ap=gt[:, :cn * D].rearrange("p (t f) -> p t f", f=D),
                    in_ap=y_own[:],
                    idxs_ap=idx_t[:, t * 8:(t + cn) * 8],
                    num_idxs=cn * 128, num_idxs_reg=cn * 128, elem_size=D)
                gv = gt[:, :cn * D].rearrange("p (t f) -> p t f", f=D)
                for tl in range(cn):
                    tg = t + tl
                    jj = tile2slot[tg]
                    first = (tg == tstart[jj])
                    if first:
                        if jj > 0:
                            selS = selp.tile([128, 128], bf16, name="selS",
                                             tag="sel")
                            nc.vector.tensor_scalar(
                                selS[:], iota[:], dsp_t[:, jj - 1:jj],
                                None, ALU.is_equal)
                            nc.tensor.matmul(ps_map[jj - 1][:], lhsT=selS[:],
                                             rhs=gv[:, tl, :],
                                             start=False, stop=True)
                            pt_last = evict_win(jj - 1, ps_map.pop(jj - 1))
                        ps_map[jj] = cvps.tile([128, D], fp32, name="cvw",
                                               tag="cv")
                    sel = selp.tile([128, 128], bf16, tag="sel")
                    nc.vector.tensor_scalar(sel[:], iota[:], dst_t[:, tg:tg + 1],
                                            None, ALU.is_equal)
                    last = (jj == NJW - 1 and tg == T_total - 1)
                    nc.tensor.matmul(ps_map[jj][:], lhsT=sel[:], rhs=gv[:, tl, :],
                                     start=first, stop=last)
                    if last:
                        pt_last = evict_win(jj, ps_map.pop(jj))
                t += cn

            # ---------------- ReduceScatter of the partial aggregates
            nc.gpsimd.collective_compute(
                "ReduceScatter", ALU.add, replica_groups=grp,
                ins=[partial[:]], outs=[agg[:]])

            # ---------------- sentence branch (gated to run under the RS)
            HsT = opool.tile([128, 6, GPC], fp32, tag="HsT")
            for b in range(GPC):
                ps_ht = typs.tile([128, B], fp32, name="tinyps", tag="tiny")[:, 0:GPC]
                hts = []
                for hsrc in (I["lasth"], I["firsth"]):
                    for sc in range(4):
                        ht = hspool.tile([128, E], bf16, name="ht", tag="ht")
                        if b == 0:
                            nc.vector.tensor_copy(ht[0:1, 0:1], pt_last[0:1, 0:1])
                        nc.sync.dma_start(
                            ht[:], hsrc[b * S + sc * 128:b * S + (sc + 1) * 128, :])
                        hts.append(ht)
                for m in range(6):
                    for i, ht in enumerate(hts):
                        nc.tensor.matmul(
                            ps_ht[:, m:m + 1],
                            lhsT=ht[:, m * 128:(m + 1) * 128],
                            rhs=onescol[:],
                            start=(i == 0), stop=(i == 7))
                nc.vector.tensor_scalar(
                    HsT[:, :, b], ps_ht[:, 0:6],
                    1.0 / (2 * S), None, ALU.mult)
            tap("hsT", HsT[:])

            # ---------------- conv tail, two phases:
            # A: per window, agg + self-loop, dinv_d scale, transpose into a
            #    full feature-major buffer (aw loads issued from the Pool
            #    queue: they sit right behind the RS, so the sentence loads
            #    on the sync queue are not blocked).
            aggTT = big.tile([128, 3, NPC], bf16, tag="big")
            for w in range(NW):
                aw = ppool.tile([128, D], bf16, tag="aw")
                nc.gpsimd.dma_start(aw[:], agg[w * 128:(w + 1) * 128, :])
                asum = ppool.tile([128, D], bf16, tag="asum")
                nc.vector.tensor_tensor(asum[:], aw[:], ybuf[:, w, :], ALU.add)
                nc.vector.tensor_scalar(asum[:], asum[:], dinv[:, w:w + 1],
                                        None, ALU.mult)
                for m in range(3):
                    trp = trps.tile([128, 128], bf16, tag="tr")
                    nc.tensor.transpose(trp[:], asum[:, m * 128:(m + 1) * 128],
                                        ident[:])
                    seg_out = aggTT[:, m, w * 128:(w + 1) * 128]
                    if m == 1:
                        nc.scalar.activation(seg_out, trp[:], AF.Copy)
                    else:
                        nc.vector.tensor_copy(seg_out, trp[:])
            # B: chunked Wc' matmuls + u-term + relu + stats (mlp-style)
            convT = big.tile([128, 3, NPC], bf16, tag="big")
            csums = spool.tile([128, 3, NCH], fp32, tag="cv_sc")
            csqs = spool.tile([128, 3, NCH], fp32, tag="cv_qc")
            for jc in range(NCH):
                sl = slice(jc * NCOL, (jc + 1) * NCOL)
                for m in range(3):
                    ps = mmps.tile([128, NCOL], fp32, tag="mm")
                    for k in range(3):
                        nc.tensor.matmul(
                            ps[:], lhsT=wcf[:, k, m * 128:(m + 1) * 128],
                            rhs=aggTT[:, k, sl], start=(k == 0), stop=False)
                    nc.tensor.matmul(
                        ps[:], lhsT=twcT[:, m * 128:(m + 1) * 128],
                        rhs=u_rowh[:, sl], start=False, stop=True)
                    nc.scalar.activation(convT[:, m, sl], ps[:], AF.Relu,
                                         bias=biasc[:, m:m + 1],
                                         accum_out=csums[:, m, jc:jc + 1])
                    sq = sqpool.tile([128, NCOL], bf16, name="sqscr2",
                                     tag="sqscr")
                    if m == 0:
                        nc.scalar.activation(sq[:], convT[:, m, sl], AF.Square,
                                             accum_out=csqs[:, m, jc:jc + 1])
                    else:
                        nc.vector.tensor_mul(sq[:], convT[:, m, sl],
                                             convT[:, m, sl])
                        nc.vector.reduce_sum(csqs[:, m, jc:jc + 1], sq[:],
                                             axis=AX.X)
            if stage <= 3.9:
                finish_early(convT[0:C, 0, 0:GPC])
                continue
            st4 = spool.tile([128, 6], fp32, tag="st4")
            for m in range(3):
                nc.vector.reduce_sum(st4[:, 2 * m:2 * m + 1], csums[:, m, :], axis=AX.X)
                nc.vector.reduce_sum(st4[:, 2 * m + 1:2 * m + 2], csqs[:, m, :], axis=AX.X)
            tap("convT", convT[:])
            red4, _ = gather_stats(st4[:], 6, "ar4")
            gp4, bp4 = bn_coeffs(red4, 3, N, "g4", "be4", "bn4")
            wp1f, biasp1 = fold_bn(W["wp1"], gp4, bp4, "bp1", "f4")
            if stage <= 4:
                finish_early(convT[0:C, 0, 0:GPC])
                continue

            # ---------------- post MLPs
            p1, st5 = mlp_layer(convT, wp1f, biasp1, "l5")
            red5, _ = gather_stats(st5[:], 6, "ar5")
            gp5, bp5 = bn_coeffs(red5, 3, N, "g5", "be5", "bn5")
            wp2f, biasp2 = fold_bn(W["wp2"], gp5, bp5, "bp2", "f5")

            p2, st6 = mlp_layer(p1, wp2f, biasp2, "l6")

            # combined final sync: BN6 stats + raw masked p2 + raw H_sent in
            # ONE AllGather; every core then finishes the (tiny) head for all
            # 64 graphs locally — no further collectives.
            pay = opool.tile([128, 102], fp32, tag="pay")
            nc.vector.tensor_copy(pay[:, 0:6], st6[:])
            for m in range(3):
                src_ap = p2[:, m, :].rearrange("p (g n) -> p g n", g=GPC)[:, :, 0:2]
                dst_ap = pay[:, 6 + 16 * m:6 + 16 * (m + 1)].rearrange(
                    "p (g n) -> p g n", n=2)
                nc.vector.tensor_copy(dst_ap, src_ap)
            nc.vector.tensor_copy(pay[:, 54:102],
                                  HsT[:].rearrange("p m b -> p (m b)"))
            red6, all6 = gather_stats(pay[:], 102, "ar6")
            gp6, bp6 = bn_coeffs(red6[:, 0:6], 3, N, "g6", "be6", "bn6")
            if stage <= 5:
                finish_early(red6[0:C, 0:6], width=6)
                continue

            # all-graph masked nodes [128, 3, 2B], BN6 applied; col = 2*graph+j
            flT = opool.tile([128, 3, 2 * B], bf16, tag="flT")
            for m in range(3):
                src_ap = all6[:, :, 6 + 16 * m:6 + 16 * (m + 1)]
                dst_ap = flT[:, m, :].rearrange("p (g c) -> p g c", c=16)
                nc.vector.tensor_scalar(dst_ap, src_ap,
                                        gp6[:, m:m + 1], bp6[:, m:m + 1],
                                        ALU.mult, ALU.add)
            tap("flT", flT[:])
            HsAll = opool.tile([128, 6, B], fp32, tag="HsAll")
            nc.vector.tensor_copy(
                HsAll[:].rearrange("p m (g b) -> p m g b", g=NCORES),
                all6[:, :, 54:102].rearrange("p g (m b) -> p m g b", b=GPC))

            # ---------------- local tail for all 64 graphs
            outcT = opool.tile([128, 6, B], fp32, tag="outcT")
            for m in range(6):
                ps_o = typs.tile([128, B], fp32, tag="tiny")
                for k in range(6):
                    kc, kj = k % 3, k // 3
                    nc.tensor.matmul(
                        ps_o[:], lhsT=wcat[:, k, m * 128:(m + 1) * 128],
                        rhs=flT[:, kc, kj::2], start=(k == 0), stop=(k == 5))
                nc.scalar.activation(outcT[:, m, :], ps_o[:], AF.Relu,
                                     bias=vcol("bcat")[:, m:m + 1])
            stt = spool.tile([128, 24], fp32, tag="stt")
            for m in range(6):
                nc.vector.reduce_sum(stt[:, 2 * m:2 * m + 1], outcT[:, m, :], axis=AX.X)
                sq = spool.tile([128, B], fp32, tag="ttsq")
                nc.scalar.square(sq[:], outcT[:, m, :])
                nc.vector.reduce_sum(stt[:, 2 * m + 1:2 * m + 2], sq[:], axis=AX.X)
                nc.vector.reduce_sum(stt[:, 12 + 2 * m:13 + 2 * m], HsAll[:, m, :], axis=AX.X)
                nc.scalar.square(sq[:], HsAll[:, m, :])
                nc.vector.reduce_sum(stt[:, 13 + 2 * m:14 + 2 * m], sq[:], axis=AX.X)
            gpc_, bpc_ = bn_coeffs(stt[:, 0:12], 6, B, "gc0", "bc0", "bnc")
            gph, bph = bn_coeffs(stt[:, 12:24], 6, B, "gc1", "bc1", "bnh")
            attT = opool.tile([128, 6, B], fp32, tag="attT")
            for m in range(6):
                nc.vector.tensor_scalar(attT[:, m, :], HsAll[:, m, :],
                                        gph[:, m:m + 1], bph[:, m:m + 1],
                                        ALU.mult, ALU.add)
                nc.vector.tensor_scalar(outcT[:, m, :], outcT[:, m, :],
                                        gpc_[:, m:m + 1], bpc_[:, m:m + 1],
                                        ALU.mult, ALU.add)
                nc.vector.tensor_add(attT[:, m, :], attT[:, m, :], outcT[:, m, :])
            ps_ft = typs.tile([128, B], fp32, tag="tiny")
            ps_f = ps_ft[0:C, :]
            for k in range(6):
                nc.tensor.matmul(ps_f, lhsT=wout[:, k, :], rhs=attT[:, k, :],
                                 start=(k == 0), stop=False)
            nc.tensor.matmul(ps_f, lhsT=brow[0:1, D:D + C], rhs=onesB[:],
                             start=False, stop=True)
            fin = spool.tile([C, B], fp32, tag="fin")
            nc.vector.tensor_copy(fin[:], ps_f)
            nc.sync.dma_start(outT[:], fin[:])

    nc.compile()
    return nc, tap_outs


# ---------------------------------------------------------------- entry
_CACHE = {}


def _get_compiled(meta):
    key = meta
    if key not in _CACHE:
        nc, _ = build(meta)
        split_waits(nc)
        _CACHE[key] = nc
    return _CACHE[key]


def kernel(**inputs):
    in_maps, meta = preprocess(inputs)
    nc = _get_compiled(meta)
    from concourse import bass2jax
    results = bass2jax.run_bass_via_pjrt(nc, in_maps, n_cores=NCORES)
    out = np.concatenate(
        [results[c]["outT"].T[c * GPC:(c + 1) * GPC] for c in range(NCORES)],
        axis=0)
    return out.astype(np.float32)
